# revision 45
# baseline (speedup 1.0000x reference)
"""Chamfer distance (squared-L2, mean of both directional min-means) on 8
Trainium2 NeuronCores — symmetric single-matmul variant.

Sharding: B=16 batches of N=M=4096 3-D points, data-parallel, 2 batches per
core.  Each batch's distance matrix is computed ONCE:
  * dist1 (min over columns for each row)   = row-max of -D
  * dist2 (min over rows for each column)   = col-max of -D
The DVE (the only engine that can do tensor-tensor min/max on this part:
GpSimd elementwise is rejected by walrus on NC v3, TensorTensorReduce and
custom-ISA fused reduce ops crash this runtime, tensor_tensor_scan is a
1-elem/cycle serial recurrence, and pool max costs a ~2k-cycle op-type
switch penalty when interleaved with tensor_tensor) is the bottleneck at
~2 touches/element in its fp16 2x mode — the read-bound floor for two
independent reductions built from 2-input max ops.

Device kernel, default "tree4" variant (GRP=4 chunks per group):
  * One K=32 stacked bf16 matmul per 128-row chunk emits negated distance
    tiles -D[n, m] into fp32 PSUM ([128, 2048] stripes, double-buffered),
    with hi/mid/lo bf16 coordinate splitting: fp32-accurate distances.
    2048-wide stripes halve ScalarE's per-op overhead vs 1024 (ScalarE is
    near co-critical; this alone was worth ~6%).
  * ScalarE cast-copies each stripe into an fp16 [128, GRP, 4096] SBUF
    group tile shared by GRP=4 chunks (4 group buffers in flight;
    A/B-measured better than 3 bufs and than GRP=8 with 2 bufs).
  * VectorE, per group (fp16 2x tensor_tensor throughout, no op-type
    switches in the hot loop):
      - col-max accumulate acc = max(acc, q) per chunk (chunks 0+1
        initialize acc = max(q0, q1) directly);
      - row-max halving tree IN-PLACE inside the group tile, each level one
        op spanning all GRP chunks: 2048 <- 1024 <- ... <- 2*TW, final
        level writes a persistent [128, 64, TW=128] batching tile (t4).
  * Per batch: acc is transposed 128x128-tile-wise (identity matmul) into
    fp16 PSUM and tensor_reduce'd into per-column maxima cm (2 groups of
    16 tiles); one tensor_reduce turns t4's batch half into rm.  All
    tensor_reduce ops sit at batch boundaries so only ~2 op-type switches
    per batch hit the DVE, and batch 0's reduces hide under batch 1's
    chunk stream.  (An all-TT finalize via ScalarE-copied transposes +
    2x halving trees, KM_FIN=ttfold, A/B-measured ~15us WORSE than the
    1x reduces — small-FD DVE ops cost more than their modeled cycles.)
  * rm/cm are fp16 (max of fp16 inputs is exact in fp16).

The host negates rm/cm (restoring +dist mins), clamps at zero (identical to
the reference's maximum(d, 0): clamping commutes with min) and averages in
f64.

Measured (slope method, R_hi=1200, interleaved rounds, min): ~281-305 us
depending on machine-state vs ~336-390 us for the tree2@1024 baseline on
the same states (~8-10%).  Rel err vs fp32 jax reference ~6e-6 (tolerance
2e-2).

Variant/knob env vars (defaults are the shipped config): KM_VARIANT=tree4,
KM_GRP=4, KM_QBUFS=4, KM_TW=128, KM_PPSW=2048, KM_FIN=reduce;
tree2/tree/pair/flat/pool/pp/prow retained for A/B timing.
"""

import os
import sys
from contextlib import ExitStack

import numpy as np

sys.path.insert(0, "/opt/trn_rl_repo")

import ml_dtypes

import concourse.bass as bass
import concourse.tile as tile
from concourse import bacc, mybir
from concourse.bass_utils import run_bass_kernel_spmd

B, N, M = 16, 4096, 4096
NCORES = 8
BPC = B // NCORES          # batches per core
K = 32                     # stacked contraction rows
NCHUNK = N // 128          # 32 output-row chunks per batch
HALF = 2048                # half-stripe width (4 PSUM banks)
BF16 = ml_dtypes.bfloat16
NEG_BIG = -60000.0         # fp16-safe "-inf" for max-accumulators
# Row-max reduction variant (see bench.py):
#   tree2: chunk PAIRS share one cast tile; each tree level is one
#          tensor_tensor spanning both chunks (fewest DVE instructions)
#   tree : colacc + 3-level tensor_tensor halving + batched reduce
#   pair : colacc + 1 halving + reduce(2048)
#   flat : colacc + single reduce(4096)
#   pp   : like tree but col-max accumulator ping-pongs (not in place)
#   notree/nocol: timing probes (partial outputs invalid)
VARIANT = os.environ.get("KM_VARIANT", "tree4")
# GpSimd col-max offload (DEAD: walrus rejects TensorTensor/TensorScalarPtr
# on the Pool engine for NeuronCore V3 — kept for reference).  0 = off.
GPD = int(os.environ.get("KM_GPD", "0"))
# Batch the per-chunk final row-max reduces: one tensor_reduce per RB chunks.
RB = int(os.environ.get("KM_RB", "8"))
QBUFS = int(os.environ.get("KM_QBUFS", "4"))
TBUFS = int(os.environ.get("KM_TBUFS", "2"))
# PSUM stripe width: 2048 (2 stripes x 2 bufs) or 1024 (4 stripes x 4 bufs)
PSW = int(os.environ.get("KM_PSW", "1024"))
# prow variant: PSUM stripe width (separate knob)
PPSW = int(os.environ.get("KM_PPSW", "2048"))
# tree4: chunks per tree group (2, 4 or 8)
GRP = int(os.environ.get("KM_GRP", "4"))
# tree4: t4 batching-tile width per chunk (128 or 256); 128 adds one tree
# level but shrinks the final reduce and frees 16KB SBUF for a 4th qp buf
TW = int(os.environ.get("KM_TW", "128"))
# tree4 finalize style: ttfold (2x TT halving trees via ScalarE-copied
# transposes) or reduce (1x tensor_reduce per batch, the older scheme)
FIN = os.environ.get("KM_FIN", "reduce")


# ----------------------------------------------------------------- host prep

def _splitn(x, n):
    """x (fp32/fp64) -> n bf16 arrays p_i with sum(p_i) = x + O(2^-(8n) x)."""
    parts = []
    r = x
    for _ in range(n):
        p = r.astype(BF16)
        parts.append(p)
        r = r - p.astype(x.dtype)
    return parts


def _stacks(z):
    """z: [N, 3] fp32 points -> (lhsT_stack [K, N] bf16, rhs_stack [K, N] bf16).

    Row pairing (lhsT row k multiplies rhs row k, summed over k): the 3-way
    bf16 split of each coordinate (h/m/l) keeps all cross products except
    l.l (2^-32 relative); |z|^2 enters as a 4-way bf16 split against a
    ones-row on the opposite side.  lhsT is globally negated so PSUM
    accumulates -D.
      k 0-8  : (-2 h1).(h2|m2|l2)    k 9-17 : (-2 m1).(h2|m2|l2)
      k 18-23: (-2 l1).(h2|m2)       k 24-27: sq1 parts . 1
      k 28-31: 1 . sq2 parts
    """
    zt = np.ascontiguousarray(z.T.astype(np.float32))          # [3, N]
    h, m, l = _splitn(zt, 3)
    sq = (z.astype(np.float64) ** 2).sum(axis=-1)              # [N]
    sqp = _splitn(sq, 4)
    npts = z.shape[0]

    lhs = np.empty((K, npts), dtype=BF16)
    h2 = (-2.0 * h.astype(np.float32)).astype(BF16)            # exact (power of 2)
    m2 = (-2.0 * m.astype(np.float32)).astype(BF16)
    l2 = (-2.0 * l.astype(np.float32)).astype(BF16)
    for i, a in enumerate((h2, h2, h2, m2, m2, m2, l2, l2)):
        lhs[3 * i: 3 * i + 3] = a
    for i in range(4):
        lhs[24 + i] = sqp[i]
    lhs[28:32] = np.ones((4, npts), dtype=BF16)

    rhs = np.empty((K, npts), dtype=BF16)
    for i, a in enumerate((h, m, l, h, m, l, h, m)):
        rhs[3 * i: 3 * i + 3] = a
    rhs[24:28] = np.ones((4, npts), dtype=BF16)
    for i in range(4):
        rhs[28 + i] = sqp[i]
    return -lhs, rhs           # negated: PSUM accumulates -D, reduce is max


# -------------------------------------------------------------- device build

def _build_nc(repeat=1):
    """repeat > 1 builds a timing variant: the full compute loop re-executes
    `repeat` times inside one NEFF (same data, idempotent: max-accumulators
    are absorbing) so per-pass hardware time can be extracted from the
    wall-clock slope."""
    nc = bacc.Bacc("TRN2", target_bir_lowering=False, debug=False)
    lhs_d = nc.dram_tensor("lhs", [BPC, K, N], mybir.dt.bfloat16,
                           kind="ExternalInput")
    rhs_d = nc.dram_tensor("rhs", [BPC, K, M], mybir.dt.bfloat16,
                           kind="ExternalInput")
    eye_d = nc.dram_tensor("eye", [128, 128], mybir.dt.float16,
                           kind="ExternalInput")
    res_dt = (mybir.dt.float16 if VARIANT in ("prow", "tree4")
              else mybir.dt.float32)
    rm_d = nc.dram_tensor("rowmax", [128, BPC * NCHUNK], res_dt,
                          kind="ExternalOutput")
    cm_d = nc.dram_tensor("colmax", [128, BPC * (M // 128)], res_dt,
                          kind="ExternalOutput")
    lhs_ap, rhs_ap = lhs_d.ap(), rhs_d.ap()

    with tile.TileContext(nc) as tc, ExitStack() as ctx:
        stacks = ctx.enter_context(tc.tile_pool(name="stacks", bufs=1))
        psw = PPSW if VARIANT in ("prow", "tree4") else PSW
        psum = ctx.enter_context(
            tc.tile_pool(name="psum", bufs=4096 // psw, space="PSUM"))
        qpool = ctx.enter_context(tc.tile_pool(name="qcast", bufs=QBUFS))
        tpool = ctx.enter_context(tc.tile_pool(name="tree", bufs=TBUFS))
        apool = ctx.enter_context(tc.tile_pool(name="accs", bufs=1))
        rpool = ctx.enter_context(tc.tile_pool(name="res", bufs=1))

        lhs_t, rhs_t = [], []
        for b in range(BPC):
            lt = stacks.tile([K, N], mybir.dt.bfloat16, tag=f"lhs{b}")
            nc.sync.dma_start(lt[:], lhs_ap[b])
            rt = stacks.tile([K, M], mybir.dt.bfloat16, tag=f"rhs{b}")
            nc.sync.dma_start(rt[:], rhs_ap[b])
            lhs_t.append(lt)
            rhs_t.append(rt)
        eye_t = stacks.tile([128, 128], mybir.dt.float16, tag="eye")
        nc.sync.dma_start(eye_t[:], eye_d.ap())

        rm = rpool.tile([128, BPC * NCHUNK], res_dt, tag="rm")
        cm = rpool.tile([128, BPC * (M // 128)], res_dt, tag="cm")
        ping_pong = VARIANT == "pp"
        acc_bufs = 2 if ping_pong else 1
        accs = []
        for b in range(BPC):
            bb = []
            for i in range(acc_bufs):
                acc = apool.tile([128, M], mybir.dt.float16,
                                 tag=f"acc{b}_{i}", name=f"acc{b}_{i}")
                nc.gpsimd.memset(acc[:], NEG_BIG)
                bb.append(acc)
            accs.append(bb)
        gaccs = []
        if GPD:
            for b in range(BPC):
                gacc = apool.tile([128, M], mybir.dt.float16,
                                  tag=f"gacc{b}", name=f"gacc{b}")
                nc.gpsimd.memset(gacc[:], NEG_BIG)
                gaccs.append(gacc)

        def body4():
            """tree4: all-TT kernel.  GRP chunks share one SBUF group tile;
            the row-max halving tree runs IN-PLACE inside it, last level
            into a persistent [128, 64, TW] batching tile (t4).  The
            finalize is also pure tensor_tensor: acc is transposed
            128x128-tile-wise into fp16 PSUM, ScalarE (never binding)
            copies the transposed tiles into an SBUF fold tile, and both
            rm (from t4) and cm (from fold) come out of in-place 2x TT
            halving trees — zero 1x tensor_reduce ops and zero DVE op-type
            switches.  Batch 0's fold trees are emitted after batch 1's
            first group so their ScalarE/TensorE inputs are long ready when
            the DVE reaches them."""
            t4 = rpool.tile([128, BPC * NCHUNK, TW], mybir.dt.float16,
                            tag="t4big", name="t4big")
            fold = (rpool.tile([128, 32, 128], mybir.dt.float16,
                               tag="fold", name="fold")
                    if FIN == "ttfold" else None)

            def emit_transposes(b):
                # TensorE + ScalarE only: acc -> PSUM (transposed) -> fold
                facc = accs[b][0]
                for s in range(2):
                    pt = psum.tile([128, 16, 128], mybir.dt.float16,
                                   tag="ps")
                    for j in range(16):
                        nc.tensor.transpose(
                            pt[:, j],
                            facc[:, (s * 16 + j) * 128:
                                 (s * 16 + j + 1) * 128],
                            eye_t[:])
                    nc.scalar.copy(fold[:, s * 16:(s + 1) * 16, :], pt[:])

            def tt_fold(src, dst2d, width):
                # in-place TT halving tree over [128, 32, width] -> dst2d
                w = width // 2
                while w >= 2:
                    nc.vector.tensor_tensor(
                        src[:, :, 0:w], src[:, :, 0:w], src[:, :, w:2 * w],
                        mybir.AluOpType.max)
                    w //= 2
                nc.vector.tensor_tensor(
                    dst2d.rearrange("p (a w) -> p a w", w=1),
                    src[:, :, 0:1], src[:, :, 1:2], mybir.AluOpType.max)

            def emit_fold_trees(b):
                tt_fold(t4[:, b * NCHUNK:(b + 1) * NCHUNK, :],
                        rm[:, b * NCHUNK:(b + 1) * NCHUNK], TW)
                tt_fold(fold, cm[:, b * 32:(b + 1) * 32], 128)

            def emit_reduce_fin(b):
                # older finalize: 1x tensor_reduce ops
                facc = accs[b][0]
                for s in range(2):
                    pt = psum.tile([128, 16, 128], mybir.dt.float16,
                                   tag="ps")
                    for j in range(16):
                        nc.tensor.transpose(
                            pt[:, j],
                            facc[:, (s * 16 + j) * 128:
                                 (s * 16 + j + 1) * 128],
                            eye_t[:])
                    nc.vector.tensor_reduce(
                        cm[:, (b * 2 + s) * 16:(b * 2 + s + 1) * 16], pt[:],
                        axis=mybir.AxisListType.X, op=mybir.AluOpType.max)
                nc.vector.tensor_reduce(
                    rm[:, b * NCHUNK:(b + 1) * NCHUNK],
                    t4[:, b * NCHUNK:(b + 1) * NCHUNK],
                    axis=mybir.AxisListType.X, op=mybir.AluOpType.max)

            for b in range(BPC):
                lt, rt = lhs_t[b], rhs_t[b]
                acc = accs[b][0]
                for cp in range(NCHUNK // GRP):
                    if FIN == "ttfold" and b == 1 and cp == 1:
                        emit_transposes(0)
                    if FIN == "ttfold" and b == 1 and cp == 2:
                        emit_fold_trees(0)
                    qp = qpool.tile([128, GRP, N], mybir.dt.float16, tag="q")
                    for cpar in range(GRP):
                        c = GRP * cp + cpar
                        for h in range(N // PPSW):
                            ps = psum.tile([128, PPSW], mybir.dt.float32,
                                           tag="ps")
                            for j in range(PPSW // 512):
                                nc.tensor.matmul(
                                    ps[:, j * 512:(j + 1) * 512],
                                    lt[:, c * 128:(c + 1) * 128],
                                    rt[:, h * PPSW + j * 512:
                                       h * PPSW + (j + 1) * 512])
                            nc.scalar.copy(
                                qp[:, cpar, h * PPSW:(h + 1) * PPSW], ps[:])
                        if cp > 0 or cpar >= 2:
                            nc.vector.tensor_tensor(
                                acc[:], acc[:], qp[:, cpar],
                                mybir.AluOpType.max)
                        elif cpar == 1:
                            nc.vector.tensor_tensor(
                                acc[:], qp[:, 0], qp[:, 1],
                                mybir.AluOpType.max)
                    # in-place halving tree inside qp (col-accs above already
                    # consumed qp in program order on the same engine)
                    w = N // 2
                    while w > TW:
                        nc.vector.tensor_tensor(
                            qp[:, :, 0:w], qp[:, :, 0:w], qp[:, :, w:2 * w],
                            mybir.AluOpType.max)
                        w //= 2
                    g = b * NCHUNK + GRP * cp
                    nc.vector.tensor_tensor(
                        t4[:, g:g + GRP], qp[:, :, 0:TW], qp[:, :, TW:2 * TW],
                        mybir.AluOpType.max)
                if FIN != "ttfold":
                    emit_reduce_fin(b)
            if FIN == "ttfold":
                emit_transposes(1)
                emit_fold_trees(1)

        def bodyP():
            """prow: per chunk, matmuls fill PPSW-wide fp32 PSUM stripes,
            ScalarE casts them into a [128, N] fp16 SBUF tile, DVE runs one
            col-max accumulate (tensor_tensor, 2x) and one single-window
            max-pool (2x) that writes the chunk's row-max straight into the
            fp16 rm column.  Chunk 0 is cast directly into the accumulator
            (initialization for free, pool reads it there)."""
            for b in range(BPC):
                lt, rt = lhs_t[b], rhs_t[b]
                acc = accs[b][0]
                for c in range(NCHUNK):
                    first = c == 0
                    q = acc if first else qpool.tile(
                        [128, N], mybir.dt.float16, tag="q")
                    for h in range(N // PPSW):
                        ps = psum.tile([128, PPSW], mybir.dt.float32,
                                       tag="ps")
                        for j in range(PPSW // 512):
                            nc.tensor.matmul(
                                ps[:, j * 512:(j + 1) * 512],
                                lt[:, c * 128:(c + 1) * 128],
                                rt[:, h * PPSW + j * 512:
                                   h * PPSW + (j + 1) * 512])
                        nc.scalar.copy(q[:, h * PPSW:(h + 1) * PPSW], ps[:])
                    if not first:
                        nc.vector.tensor_tensor(
                            acc[:], acc[:], q[:], mybir.AluOpType.max)
                    g = b * NCHUNK + c
                    nc.vector.pool(
                        rm[:, g:g + 1],
                        q[:].rearrange("p (a w) -> p a w", w=N),
                        mybir.PoolFunctionType.max)
                _finalize(b, acc)

        def body2():
            """tree2: chunks processed in pairs sharing one [128, 2, N] cast
            tile; each tree level is a single tensor_tensor spanning both
            chunks (halved per-op overhead, fewer DVE instructions)."""
            rbp = max(RB // 2, 1)      # reduce batching in pairs
            if VARIANT == "tree2i":
                order = [(b, cp) for cp in range(NCHUNK // 2)
                         for b in range(BPC)]
            else:
                order = [(b, cp) for b in range(BPC)
                         for cp in range(NCHUNK // 2)]
            t3m = {b: [None] for b in range(BPC)}
            for b, cp in order:
                lt, rt = lhs_t[b], rhs_t[b]
                acc = accs[b][0]
                if True:
                    qp = qpool.tile([128, 2, N], mybir.dt.float16, tag="q")
                    for cpar in range(2):
                        c = 2 * cp + cpar
                        for h in range(N // PSW):
                            ps = psum.tile([128, PSW], mybir.dt.float32,
                                           tag="ps")
                            for j in range(PSW // 512):
                                nc.tensor.matmul(
                                    ps[:, j * 512:(j + 1) * 512],
                                    lt[:, c * 128:(c + 1) * 128],
                                    rt[:, h * PSW + j * 512:
                                       h * PSW + (j + 1) * 512])
                            nc.scalar.copy(
                                qp[:, cpar, h * PSW:(h + 1) * PSW], ps[:])
                        if cp > 0:
                            nc.vector.tensor_tensor(
                                acc[:], acc[:], qp[:, cpar],
                                mybir.AluOpType.max)
                    if cp == 0:
                        # first pair initializes the accumulator directly
                        nc.vector.tensor_tensor(
                            acc[:], qp[:, 0], qp[:, 1], mybir.AluOpType.max)
                    t1 = tpool.tile([128, 2, 2048], mybir.dt.float16,
                                    tag="t1")
                    nc.vector.tensor_tensor(
                        t1[:], qp[:, :, 0:HALF], qp[:, :, HALF:N],
                        mybir.AluOpType.max)
                    t2 = tpool.tile([128, 2, 1024], mybir.dt.float16,
                                    tag="t2")
                    nc.vector.tensor_tensor(
                        t2[:], t1[:, :, 0:1024], t1[:, :, 1024:2048],
                        mybir.AluOpType.max)
                    if cp % rbp == 0:
                        t3m[b][0] = tpool.tile([128, 2 * rbp, 512],
                                               mybir.dt.float16,
                                               tag=f"t3_{b}", name="t3")
                    t3 = t3m[b][0]
                    nc.vector.tensor_tensor(
                        t3[:, 2 * (cp % rbp):2 * (cp % rbp) + 2],
                        t2[:, :, 0:512], t2[:, :, 512:1024],
                        mybir.AluOpType.max)
                    if cp % rbp == rbp - 1:
                        g = b * NCHUNK + 2 * (cp - rbp + 1)
                        nc.vector.tensor_reduce(
                            rm[:, g:g + 2 * rbp], t3[:],
                            axis=mybir.AxisListType.X,
                            op=mybir.AluOpType.max)
                if cp == NCHUNK // 2 - 1:
                    _finalize(b, accs[b][0])

        def _finalize(b, facc):
            if GPD:
                nc.vector.tensor_tensor(
                    facc[:], facc[:], gaccs[b][:], mybir.AluOpType.max)
            for s in range(2):
                pt = psum.tile([128, 16, 128], mybir.dt.float16, tag="ps")
                for j in range(16):
                    nc.tensor.transpose(
                        pt[:, j],
                        facc[:, (s * 16 + j) * 128:(s * 16 + j + 1) * 128],
                        eye_t[:])
                col = (b * 2 + s) * 16
                nc.vector.tensor_reduce(
                    cm[:, col:col + 16], pt[:],
                    axis=mybir.AxisListType.X, op=mybir.AluOpType.max)

        t3s = [None]

        def body():
            for b in range(BPC):
                lt, rt = lhs_t[b], rhs_t[b]
                for c in range(NCHUNK):
                    # chunk 0 casts straight into the accumulator: the first
                    # col-max accumulate becomes a plain initialization and
                    # the row-max tree reads the same values from acc.
                    first = c == 0 and VARIANT not in ("pp", "nocol") \
                        and not GPD
                    if first:
                        q = accs[b][0]
                    else:
                        q = qpool.tile([128, N], mybir.dt.float16, tag="q")
                    for h in range(N // PSW):
                        ps = psum.tile([128, PSW], mybir.dt.float32, tag="ps")
                        for j in range(PSW // 512):
                            nc.tensor.matmul(
                                ps[:, j * 512:(j + 1) * 512],
                                lt[:, c * 128:(c + 1) * 128],
                                rt[:, h * PSW + j * 512:
                                   h * PSW + (j + 1) * 512])
                        nc.scalar.copy(q[:, h * PSW:(h + 1) * PSW], ps[:])
                    # col-max accumulate (elementwise, fp16 2x)
                    if VARIANT != "nocol" and not first:
                        if GPD and c % GPD == GPD - 1:
                            gacc = gaccs[b]
                            nc.gpsimd.tensor_tensor(
                                gacc[:], gacc[:], q[:], mybir.AluOpType.max)
                        elif ping_pong:
                            src = accs[b][c % 2]
                            dst = accs[b][(c + 1) % 2]
                            nc.vector.tensor_tensor(
                                dst[:], src[:], q[:], mybir.AluOpType.max)
                        else:
                            acc = accs[b][0]
                            nc.vector.tensor_tensor(
                                acc[:], acc[:], q[:], mybir.AluOpType.max)
                    # row-max reduction
                    g = b * NCHUNK + c
                    if VARIANT == "notree":
                        pass
                    elif VARIANT == "flat":
                        nc.vector.tensor_reduce(
                            rm[:, g:g + 1], q[:],
                            axis=mybir.AxisListType.X, op=mybir.AluOpType.max)
                    elif VARIANT == "pool":
                        # whole row-max in one DVE max-pool op
                        nc.vector.pool(
                            rm[:, g:g + 1],
                            q[:].rearrange("p (a w) -> p a w", w=N),
                            mybir.PoolFunctionType.max)
                    elif VARIANT == "pool2":
                        # halve with tensor_tensor, then one max-pool
                        t1 = tpool.tile([128, 2048], mybir.dt.float16,
                                        tag="t1")
                        nc.vector.tensor_tensor(
                            t1[:], q[:, 0:HALF], q[:, HALF:N],
                            mybir.AluOpType.max)
                        nc.vector.pool(
                            rm[:, g:g + 1],
                            t1[:].rearrange("p (a w) -> p a w", w=HALF),
                            mybir.PoolFunctionType.max)
                    elif VARIANT == "pair":
                        t1 = tpool.tile([128, 2048], mybir.dt.float16,
                                        tag="t1")
                        nc.vector.tensor_tensor(
                            t1[:], q[:, 0:HALF], q[:, HALF:N],
                            mybir.AluOpType.max)
                        nc.vector.tensor_reduce(
                            rm[:, g:g + 1], t1[:],
                            axis=mybir.AxisListType.X, op=mybir.AluOpType.max)
                    else:                  # tree / pp / nocol
                        t1 = tpool.tile([128, 2048], mybir.dt.float16,
                                        tag="t1")
                        nc.vector.tensor_tensor(
                            t1[:], q[:, 0:HALF], q[:, HALF:N],
                            mybir.AluOpType.max)
                        t2 = tpool.tile([128, 1024], mybir.dt.float16,
                                        tag="t2")
                        nc.vector.tensor_tensor(
                            t2[:], t1[:, 0:1024], t1[:, 1024:2048],
                            mybir.AluOpType.max)
                        if c % RB == 0:
                            t3s[0] = tpool.tile([128, RB, 512],
                                                mybir.dt.float16, tag="t3",
                                                name="t3")
                        t3 = t3s[0]
                        nc.vector.tensor_tensor(
                            t3[:, c % RB], t2[:, 0:512], t2[:, 512:1024],
                            mybir.AluOpType.max)
                        if c % RB == RB - 1:
                            nc.vector.tensor_reduce(
                                rm[:, g - RB + 1:g + 1], t3[:],
                                axis=mybir.AxisListType.X,
                                op=mybir.AluOpType.max)
                # finalize col-max: transpose acc tile-wise, reduce over rows
                facc = accs[b][NCHUNK % 2] if ping_pong else accs[b][0]
                if GPD:
                    nc.vector.tensor_tensor(
                        facc[:], facc[:], gaccs[b][:], mybir.AluOpType.max)
                for s in range(2):
                    pt = psum.tile([128, 16, 128], mybir.dt.float16, tag="ps")
                    for j in range(16):
                        nc.tensor.transpose(
                            pt[:, j],
                            facc[:, (s * 16 + j) * 128:(s * 16 + j + 1) * 128],
                            eye_t[:])
                    col = (b * 2 + s) * 16
                    nc.vector.tensor_reduce(
                        cm[:, col:col + 16], pt[:],
                        axis=mybir.AxisListType.X, op=mybir.AluOpType.max)

        if VARIANT == "prow":
            bfn = bodyP
        elif VARIANT == "tree4":
            bfn = body4
        elif VARIANT == "tree2":
            bfn = body2
        else:
            bfn = body
        if repeat > 1:
            with tc.For_i(0, repeat, 1):
                bfn()
        else:
            bfn()
        nc.sync.dma_start(rm_d.ap(), rm[:])
        nc.sync.dma_start(cm_d.ap(), cm[:])
    nc.compile()
    return nc


_CACHE: dict = {}


def _get_nc():
    if "nc" not in _CACHE:
        _CACHE["nc"] = _build_nc()
    return _CACHE["nc"]


# --------------------------------------------------------------------- entry

def make_in_maps(xyz1, xyz2):
    eye = np.eye(128, dtype=np.float16)
    in_maps = []
    for core in range(NCORES):
        lhs = np.empty((BPC, K, N), dtype=BF16)
        rhs = np.empty((BPC, K, M), dtype=BF16)
        for bl in range(BPC):
            b = core * BPC + bl
            ls, _ = _stacks(np.asarray(xyz1[b]))
            _, rs = _stacks(np.asarray(xyz2[b]))
            lhs[bl], rhs[bl] = ls, rs
        in_maps.append({"lhs": lhs, "rhs": rhs, "eye": eye})
    return in_maps


def combine(results):
    total = 0.0
    for r in results:
        rm = -r["rowmax"].astype(np.float64)   # [128, 64] -> dist1 mins
        cm = -r["colmax"].astype(np.float64)   # [128, 64] -> dist2 mins
        total += np.maximum(rm, 0.0).sum() + np.maximum(cm, 0.0).sum()
    return np.float32(total / (B * N))


def kernel(xyz1, xyz2, **_):
    in_maps = make_in_maps(xyz1, xyz2)
    try:
        res = run_bass_kernel_spmd(_get_nc(), in_maps,
                                   core_ids=list(range(NCORES)))
    except Exception:                      # transient axon/PJRT hiccup
        _CACHE.clear()
        res = run_bass_kernel_spmd(_get_nc(), in_maps,
                                   core_ids=list(range(NCORES)))
    return combine(res.results)

